# revision 25
# baseline (speedup 1.0000x reference)
"""Trainium2 Bass kernel for nn_MultiHeadAttention_88210038326473.

Reference computation (B=4, S=2048, HID=2048, H=16, DH=128):
    Q = queries @ Wq.T + bq ; K = keys @ Wk.T + bk ; V = keys @ Wv.T + bv
    per-head scores = Qh Kh^T / sqrt(HID), key-padding + causal mask,
    softmax, out = attn @ Vh, concat heads, + queries residual.

Sharding: 8 cores = 4 batches x 2 head-groups (8 heads each). Each core
computes out[b, :, hg*1024:(hg+1)*1024] (stored transposed [1024, 2048];
host transposes back and assembles).

Implementation: fp8(e4m3) operands with DoubleRow matmuls (2 contraction
subtiles of 128 per PE stream) for the three projections and the
attention AV / row-sum matmuls. Host prescales weights by 32 so fp8
weight values avoid the subnormal range; the 1/32 factors are folded
into the exp scale and the row-sum ones value. All of KT/VT/QT stay
SBUF-resident between projection and attention (no DRAM scratch).
Scores are computed transposed (sT[k,q]) in 2-bank PSUM pair tiles,
exp'd in one ScalarE call per pair (fp8 out), causal-masked with
precomputed sliding-window 0/1 masks on DVE, then consumed by
DoubleRow AV and row-sum matmuls. Normalization: DR ones-matmul row
sums -> DVE reciprocal -> PE rank-1 broadcast matmul (f32r) -> DVE
normalize + residual(bf16) add, deferred one q-chunk to keep PE fed.
Q-projection of head h+1 is woven between attention pairs of head h so
ScalarE exp time hides under PE matmuls.
"""

import math
import os as _osmod

import numpy as np

_osenv = _osmod.environ

B, S, HID, H, DH = 4, 2048, 2048, 16, 128
NCORES = 8
HPC = 8          # heads per core
EH = HPC * DH    # 1024 e-dims per core
SCALE = 1.0 / math.sqrt(HID)
WS = 32.0        # host-side weight scale (fp8 subnormal avoidance)
EFF_SCALE = float(SCALE / (WS * WS))  # exp scale: undo Q,K weight scaling
PC = 512         # projection s-chunk (matmul moving N)
NPC = S // PC    # 4
QC = 512         # attention q-chunk
NQC = S // QC    # 4
NKT = S // DH    # 16 k-tiles
NF = HID // DH   # 16 f-tiles (contraction)
NFP = NF // 2    # 8 f-pairs (DoubleRow)
NEG_BIAS = np.float32(-1.0e30)
COMPUTE_MAX_WAITS = int(_osenv.get('K_CMW', '1'))  # waits on non-CTRL instructions


CTRL_OPS = ("InstDrain", "InstNoOp", "InstEventSemaphore", "InstISA")


def _split_excess_waits(nc, max_waits=1, compute_max_waits=None):
    """walrus in this container rejects >1 sem-wait per CTRL-class instruction.
    Move excess waits onto preceding NoOps on the same engine. Compute-class
    instructions may support more waits (compute_max_waits)."""
    import concourse.mybir as mybir

    if compute_max_waits is None:
        compute_max_waits = max_waits
    n_split = 0
    for fn in nc.m.functions:
        for blk in fn.blocks:
            insts = list(blk.instructions)
            out = []
            changed = False
            for ins in insts:
                lim = (
                    max_waits
                    if type(ins).__name__ in CTRL_OPS
                    else compute_max_waits
                )
                si = ins.sync_info
                if si is not None and si.on_wait and len(si.on_wait) > lim:
                    waits = list(si.on_wait)
                    carriers, rest = waits[:-lim], waits[-lim:]
                    for i in range(0, len(carriers), max_waits):
                        chunk = carriers[i : i + max_waits]
                        out.append(
                            mybir.InstNoOp(
                                name=f"{ins.name}-ws{i}",
                                engine=ins.engine,
                                bass_nofuse=True,
                                sync_info=mybir.SyncInfo(on_wait=chunk, on_update=[]),
                            )
                        )
                        n_split += 1
                    ins.sync_info = mybir.SyncInfo(
                        on_wait=rest, on_update=list(si.on_update)
                    )
                    changed = True
                out.append(ins)
            if changed:
                blk.instructions = out
    return n_split


_CACHE = {}


def _build(fast=True, zero_bias=True, phases=("kv", "qattn"), reps=1,
           act_dt=None, scale=None):
    """Build the (core-uniform) Bass program. Returns nc.

    fast=True drops the key-padding bias from the exp (valid when no key is
    padding -- checked on host). zero_bias=True skips bias adds (all-zero
    biases, checked on host). reps/scale repeat phases for timing
    instrumentation. act_dt accepted for interface compat (ignored; fp8)."""
    scale = scale or {}
    key = ("nc", fast, zero_bias, tuple(phases), reps,
           tuple(sorted(scale.items())))
    if key in _CACHE:
        return _CACHE[key]

    import concourse.bass as bass
    import concourse.mybir as mybir
    from concourse.tile import TileContext

    F32 = mybir.dt.float32
    F32R = mybir.dt.float32r
    BF16 = mybir.dt.bfloat16
    FP8 = mybir.dt.float8e4
    EXP = mybir.ActivationFunctionType.Exp
    IDENT = mybir.ActivationFunctionType.Identity
    DR = mybir.MatmulPerfMode.DoubleRow

    nc = bass.Bass("TRN2", target_bir_lowering=False, debug=False)

    qT = nc.dram_tensor("qT", [HID, S], FP8, kind="ExternalInput")
    kT = nc.dram_tensor("kT", [HID, S], FP8, kind="ExternalInput")
    wqT = nc.dram_tensor("wqT", [HID, EH], FP8, kind="ExternalInput")
    wkT = nc.dram_tensor("wkT", [HID, EH], FP8, kind="ExternalInput")
    wvT = nc.dram_tensor("wvT", [HID, EH], FP8, kind="ExternalInput")
    bq_d = nc.dram_tensor("bq_d", [DH, HPC], F32, kind="ExternalInput")
    bk_d = nc.dram_tensor("bk_d", [DH, HPC], F32, kind="ExternalInput")
    bv_d = nc.dram_tensor("bv_d", [1, EH], FP8, kind="ExternalInput")
    kbias_d = nc.dram_tensor("kbias_d", [DH, NKT], F32, kind="ExternalInput")
    wins_d = nc.dram_tensor("wins_d", [DH, 4 * QC], FP8, kind="ExternalInput")
    ones16_d = nc.dram_tensor("ones16_d", [DH, 32], FP8, kind="ExternalInput")
    onesr32_d = nc.dram_tensor("onesr32_d", [1, DH], F32R, kind="ExternalInput")
    onesr8_d = nc.dram_tensor("onesr8_d", [1, DH], FP8, kind="ExternalInput")
    resid_d = nc.dram_tensor("resid_d", [EH, S], BF16, kind="ExternalInput")
    outT_d = nc.dram_tensor("outT_d", [EH, S], F32, kind="ExternalOutput")

    # 3D views with the 128-partition dim innermost on rows
    qT3 = qT[:].rearrange("(f p) s -> p f s", p=DH)
    kT3 = kT[:].rearrange("(f p) s -> p f s", p=DH)
    wq3 = wqT[:].rearrange("(f p) e -> p f e", p=DH)
    wk3 = wkT[:].rearrange("(f p) e -> p f e", p=DH)
    wv3 = wvT[:].rearrange("(f p) e -> p f e", p=DH)

    ctx = dict(
        F32=F32, F32R=F32R, BF16=BF16, FP8=FP8, EXP=EXP, IDENT=IDENT, DR=DR,
        fast=fast, zero_bias=zero_bias, scale=scale,
        qT3=qT3, kT3=kT3, wq3=wq3, wk3=wk3, wv3=wv3,
        resid_d=resid_d, outT_d=outT_d,
    )

    with TileContext(nc) as tc, nc.allow_low_precision(reason="fp8 attn"):
        with tc.tile_pool(name="persist", bufs=1) as persist:
            kres = persist.tile([DH, HPC * S], FP8, tag="kres")
            qres = persist.tile([DH, HPC * S], FP8, tag="qres")
            vres = persist.tile([DH, NKT * EH], FP8, tag="vres")
            wins = persist.tile([DH, 4 * QC], FP8, tag="wins")
            ones16 = persist.tile([DH, 32], FP8, tag="ones16")
            onesr32 = persist.tile([1, DH], F32R, tag="onesr32")
            onesr8 = persist.tile([1, DH], FP8, tag="onesr8")
            bq_sb = persist.tile([DH, HPC], F32, tag="bq")
            bk_sb = persist.tile([DH, HPC], F32, tag="bk")
            bv_sb = persist.tile([1, EH], FP8, tag="bv")
            kbias = persist.tile([DH, NKT], F32, tag="kbias")
            nc.sync.dma_start(wins[:], wins_d[:])
            nc.sync.dma_start(ones16[:], ones16_d[:])
            nc.sync.dma_start(onesr32[:], onesr32_d[:])
            nc.sync.dma_start(onesr8[:], onesr8_d[:])
            nc.sync.dma_start(bq_sb[:], bq_d[:])
            nc.sync.dma_start(bk_sb[:], bk_d[:])
            nc.sync.dma_start(bv_sb[:], bv_d[:])
            nc.sync.dma_start(kbias[:], kbias_d[:])

            ctx.update(
                kres3=kres[:].rearrange("p (h s) -> p h s", h=HPC),
                qres3=qres[:].rearrange("p (h s) -> p h s", h=HPC),
                vres3=vres[:].rearrange("p (kt e) -> p kt e", kt=NKT),
                wins=wins, ones16=ones16, onesr32=onesr32, onesr8=onesr8,
                bq_sb=bq_sb, bk_sb=bk_sb, bv_sb=bv_sb, kbias=kbias,
            )

            for _rep in range(reps):
                _rep_body(nc, tc, phases, ctx)

    _split_excess_waits(nc, max_waits=1, compute_max_waits=COMPUTE_MAX_WAITS)
    _CACHE[key] = nc
    return nc


import os as _os

TRI_ENGINE = _os.environ.get("K_TRI_ENGINE", "vector")
ADD_ENGINE = _os.environ.get("K_ADD_ENGINE", "vector")


def _tri_eng(nc):
    return nc.gpsimd if TRI_ENGINE == "gpsimd" else nc.vector


def _add_eng(nc):
    return nc.gpsimd if ADD_ENGINE == "gpsimd" else nc.vector


def _proj_copy(nc, ctx, dst, psrc, bias_col, engine="vector"):
    """PSUM [DH, 512] f32 -> SBUF fp8, optional per-partition bias."""
    if ctx["zero_bias"]:
        if engine == "scalar":
            nc.scalar.copy(dst, psrc)
        else:
            nc.vector.tensor_copy(dst, psrc)
    else:
        nc.scalar.activation(dst, psrc, ctx["IDENT"], bias=bias_col)


def _rep_body(nc, tc, phases, ctx):
    DR = ctx["DR"]
    F32 = ctx["F32"]
    F32R = ctx["F32R"]
    FP8 = ctx["FP8"]
    scale = ctx["scale"]

    # Wave-structured schedule: for each 512-token chunk c, project K/V for
    # that chunk, then run every head's attention q-chunk c (which only needs
    # k-tiles <= 4c+3, i.e. chunks <= c). This interleaves the ScalarE-heavy
    # exp work with the PE-heavy projections across the whole timeline.
    # PSUM (8 banks): pp-slots 2x2 (scores pairs + K/V/Q proj tiles + the
    # tail broadcast), po 2, psums 1, pbc 1.
    with tc.tile_pool(name="wk", bufs=1) as wkp, \
         tc.tile_pool(name="wv", bufs=1) as wvp, \
         tc.tile_pool(name="wq", bufs=1) as wqp, \
         tc.tile_pool(name="qall", bufs=1) as qallp, \
         tc.tile_pool(name="kc", bufs=3) as kcp, \
         tc.tile_pool(name="ex", bufs=4) as expp, \
         tc.tile_pool(name="tailsb", bufs=3) as tailsb, \
         tc.tile_pool(name="bcs", bufs=2) as bcsp, \
         tc.tile_pool(name="rsd", bufs=3) as rsdp, \
         tc.tile_pool(name="oth", bufs=3) as othp, \
         tc.tile_pool(name="psm", bufs=2, space="PSUM") as psm, \
         tc.tile_pool(name="po", bufs=2, space="PSUM") as pop, \
         tc.tile_pool(name="psums", bufs=1, space="PSUM") as psumsp, \
         tc.tile_pool(name="pbc", bufs=1, space="PSUM") as pbcp:
        wk_t = wkp.tile([DH, NF * EH], FP8, tag="wk", name="wk")
        wk3t_w = wk_t[:].rearrange("p (f e) -> p f e", f=NF)
        # first e-half of Wk only: the first K matmuls (et 0-3) start after
        # 2 MB of critical DMA instead of 5 MB
        nc.sync.dma_start(wk3t_w[:, :, 0 : EH // 2], ctx["wk3"][:, :, 0 : EH // 2])
        wv_t = wvp.tile([DH, NF * EH], FP8, tag="wv", name="wv")
        wq_t = wqp.tile([DH, NF * EH], FP8, tag="wq", name="wq")
        qall = qallp.tile([DH, NF * S], FP8, tag="qall", name="qall")
        qall3 = qall[:].rearrange("p (f s) -> p f s", f=NF)
        wk3t = wk_t[:].rearrange("p (f e) -> p f e", f=NF)
        wv3t = wv_t[:].rearrange("p (f e) -> p f e", f=NF)
        wq3t = wq_t[:].rearrange("p (f e) -> p f e", f=NF)
        ones16v = ctx["ones16"][:].rearrange("p (k m) -> p k m", k=2)
        tri = ctx["wins"][:, 0:DH]

        # zero both score-PSUM slots once: band pairs exp() regions their
        # matmuls never wrote; stale PSUM must stay finite
        for _z in range(2):
            ppz = psm.tile([DH, 2 * QC], F32, tag="pp", name="ppz")
            nc.vector.memset(ppz[:], 0.0)

        def emit_kv_chunk(s0):
            kc = kcp.tile([DH, NF * PC], FP8, tag="kc", name="kc")
            kc3 = kc[:].rearrange("p (f s) -> p f s", f=NF)
            nc.sync.dma_start(kc3, ctx["kT3"][:, :, s0 : s0 + PC])
            if s0 == 0:
                # the rest of the weights + q-side loads ride behind the
                # wave-0 K-critical loads
                nc.sync.dma_start(
                    wk3t_w[:, :, EH // 2 : EH], ctx["wk3"][:, :, EH // 2 : EH]
                )
                nc.sync.dma_start(
                    wv_t[:].rearrange("p (f e) -> p f e", f=NF), ctx["wv3"]
                )
                nc.sync.dma_start(qall3[:, :, 0:PC], ctx["qT3"][:, :, 0:PC])
                nc.sync.dma_start(
                    wq_t[:].rearrange("p (f e) -> p f e", f=NF), ctx["wq3"]
                )
            else:
                nc.sync.dma_start(
                    qall3[:, :, s0 : s0 + PC], ctx["qT3"][:, :, s0 : s0 + PC]
                )
            for et in range(HPC):
                pk_full = psm.tile([DH, 2 * QC], F32, tag="pp", name="pk")
                pk = pk_full[:, 0:PC]
                for i in range(NFP):
                    nc.tensor.matmul(
                        pk,
                        wk3t[:, 2 * i : 2 * i + 2, et * DH : (et + 1) * DH],
                        kc3[:, 2 * i : 2 * i + 2, :],
                        start=(i == 0), stop=(i == NFP - 1), perf_mode=DR,
                    )
                _proj_copy(
                    nc, ctx, ctx["kres3"][:, et, s0 : s0 + PC], pk,
                    ctx["bk_sb"][:, et : et + 1], engine="scalar",
                )
            for sti in range(PC // DH):
                kt = s0 // DH + sti
                for ec in range(2):
                    pv_full = psm.tile([DH, 2 * QC], F32, tag="pp", name="pv")
                    pv = pv_full[:, 0:512]
                    for i in range(NFP):
                        nc.tensor.matmul(
                            pv,
                            kc3[:, 2 * i : 2 * i + 2,
                                sti * DH : (sti + 1) * DH],
                            wv3t[:, 2 * i : 2 * i + 2,
                                 ec * 512 : (ec + 1) * 512],
                            start=(i == 0),
                            stop=(ctx["zero_bias"] and i == NFP - 1),
                            perf_mode=DR,
                        )
                    if not ctx["zero_bias"]:
                        nc.tensor.matmul(
                            pv,
                            ctx["onesr8"][:],
                            ctx["bv_sb"][:, ec * 512 : (ec + 1) * 512],
                            start=False, stop=True,
                        )
                    nc.scalar.copy(
                        ctx["vres3"][:, kt, ec * 512 : (ec + 1) * 512], pv
                    )

        def emit_qproj(h, sc):
            pq_full = psm.tile([DH, 2 * QC], F32, tag="pp", name="pq")
            pq = pq_full[:, 0:PC]
            for i in range(NFP):
                nc.tensor.matmul(
                    pq,
                    wq3t[:, 2 * i : 2 * i + 2, h * DH : (h + 1) * DH],
                    qall3[:, 2 * i : 2 * i + 2, sc * PC : (sc + 1) * PC],
                    start=(i == 0), stop=(i == NFP - 1), perf_mode=DR,
                )
            _proj_copy(
                nc, ctx, ctx["qres3"][:, h, sc * PC : (sc + 1) * PC], pq,
                ctx["bq_sb"][:, h : h + 1],
            )

        pending_tail = [None]

        def flush_tail():
            if pending_tail[0] is not None:
                pending_tail[0]()
                pending_tail[0] = None

        # Global 1-pair software pipeline: the AV/row-sum matmuls of pair p
        # (which wait on its exp) are emitted only after the NEXT unit of
        # independent PE work (next pair's scores, a q-projection, or a KV
        # chunk), so the PE queue never stalls on a fresh exp.
        pend_B = [None]

        def pump_B():
            if pend_B[0] is not None:
                pend_B[0]()
                pend_B[0] = None

        def after_passthrough():
            pump_B()

        def emit_attn_A(inst, pi):
            h, q0, nkt = inst["h"], inst["q0"], inst["nkt"]
            kt0 = 2 * pi
            band = kt0 >= nkt - 4
            jb0 = kt0 - (nkt - 4) if band else 0
            offs = [
                (kt0 + j2 - (nkt - 4)) * DH if band else 0
                for j2 in range(2)
            ]
            pp = psm.tile([DH, 2 * QC], F32, tag="pp", name="pp")
            ppv = pp[:].rearrange("p (j q) -> p j q", j=2)
            for j2 in range(2):
                kt = kt0 + j2
                off = offs[j2]
                nc.tensor.matmul(
                    ppv[:, j2, off:QC],
                    ctx["kres3"][:, h, kt * DH : (kt + 1) * DH],
                    ctx["qres3"][:, h, q0 + off : q0 + QC],
                    start=True, stop=True,
                )
            ex = expp.tile([DH, 2 * QC], FP8, tag="ex", name="ex")
            ex3b = ex[:].rearrange("p (j q) -> p j q", j=2)
            if ctx["fast"]:
                lo = jb0 * DH
                nc.scalar.activation(
                    ex[:, lo : 2 * QC], pp[:, lo : 2 * QC],
                    ctx["EXP"], scale=EFF_SCALE,
                )
            else:
                for j2 in range(2):
                    kt = kt0 + j2
                    off = offs[j2]
                    nc.scalar.activation(
                        ex3b[:, j2, off:QC], ppv[:, j2, off:QC],
                        ctx["EXP"],
                        bias=ctx["kbias"][:, kt : kt + 1],
                        scale=EFF_SCALE,
                    )
            if band:
                # causal triangle on each diagonal 128x128 block;
                # left-of-band is never read downstream
                for j2 in range(2):
                    off = offs[j2]
                    _tri_eng(nc).tensor_mul(
                        ex3b[:, j2, off : off + DH],
                        ex3b[:, j2, off : off + DH],
                        tri,
                    )
            return dict(ex3b=ex3b, ex=ex, offs=offs, band=band, kt0=kt0)

        def emit_attn_B(inst, pi, a):
            h, nkt, npair = inst["h"], inst["nkt"], inst["npair"]
            po, psums = inst["po"], inst["psums"]
            kt0, band, offs = a["kt0"], a["band"], a["offs"]
            ex3b = a["ex3b"]
            if band:
                for j2 in range(2):
                    kt = kt0 + j2
                    off = offs[j2]
                    first = pi == 0 and j2 == 0
                    last = kt == nkt - 1
                    nc.tensor.matmul(
                        po[:, off:QC],
                        ctx["vres3"][:, kt, h * DH : (h + 1) * DH],
                        ex3b[:, j2, off:QC],
                        start=first, stop=last,
                    )
                    nc.tensor.matmul(
                        psums[:, off:QC],
                        ctx["ones16"][:, 0:1],
                        ex3b[:, j2, off:QC],
                        start=first, stop=last,
                    )
            else:
                ex3 = a["ex"][:].rearrange("p (k q) -> p k q", k=2)
                nc.tensor.matmul(
                    po[:],
                    ctx["vres3"][:, kt0 : kt0 + 2, h * DH : (h + 1) * DH],
                    ex3,
                    start=(pi == 0), stop=False,
                    perf_mode=DR,
                )
                nc.tensor.matmul(
                    psums[:],
                    ones16v[:, :, 0:1],
                    ex3,
                    start=(pi == 0), stop=False,
                    perf_mode=DR,
                )
            if pi == npair - 1:
                _complete(inst)

        def _complete(inst):
            h, q0 = inst["h"], inst["q0"]
            po, psums, oth, rsd = (inst["po"], inst["psums"], inst["oth"],
                                   inst["rsd"])
            # free psums promptly; the f32r copy feeds the tail broadcast
            sums_sb = tailsb.tile([1, QC], F32R, tag="sums", name="sums")
            nc.vector.tensor_copy(sums_sb[:], psums[:])
            flush_tail()

            def tail():
                # rank-1 broadcast of the row sums, then a full-width
                # reciprocal (same DVE cost as [1,512], all 128 lanes)
                # -> normalize + residual add -> store
                pbc = pbcp.tile([DH, QC], F32, name="pbc")
                nc.tensor.matmul(
                    pbc[:], ctx["onesr32"][:], sums_sb[:],
                    start=True, stop=True,
                )
                rec128 = bcsp.tile([DH, QC], F32, tag="bcs", name="bcs")
                nc.vector.reciprocal(rec128[:], pbc[:])
                nc.vector.tensor_mul(oth[:], po[:], rec128[:])
                _add_eng(nc).tensor_add(oth[:], oth[:], rsd[:])
                nc.sync.dma_start(
                    ctx["outT_d"][h * DH : (h + 1) * DH, q0 : q0 + QC],
                    oth[:],
                )

            pending_tail[0] = tail

        def emit_attn(h, qcI, next_qproj=None):
            q0 = qcI * QC
            nkt = 4 * (qcI + 1)
            rsd = rsdp.tile([DH, QC], ctx["BF16"], tag="rsd", name="rsd")
            nc.sync.dma_start(
                rsd[:], ctx["resid_d"][h * DH : (h + 1) * DH, q0 : q0 + QC]
            )
            inst = dict(
                h=h, q0=q0, nkt=nkt, npair=nkt // 2,
                po=pop.tile([DH, QC], F32, name="po"),
                psums=psumsp.tile([1, QC], F32, name="psums"),
                oth=othp.tile([DH, QC], F32, tag="oth", name="oth"),
                rsd=rsd,
            )
            for pi in range(inst["npair"]):
                a = emit_attn_A(inst, pi)
                pump_B()
                pend_B[0] = (lambda inst=inst, pi=pi, a=a:
                             emit_attn_B(inst, pi, a))
                if pi == 0 and next_qproj is not None:
                    emit_qproj(*next_qproj)
                    pump_B()

        kv_only = "qattn" not in phases
        attn_only = "kv" not in phases
        kv_rep = scale.get("kv", 1)
        attn_rep = scale.get("attn", 1)
        for c in range(NPC):
            if not attn_only:
                for _r in range(kv_rep):
                    emit_kv_chunk(c * PC)
                    pump_B()
            if kv_only:
                continue
            if attn_only and c == 0:
                nc.sync.dma_start(qall3, ctx["qT3"])
                nc.sync.dma_start(
                    wq_t[:].rearrange("p (f e) -> p f e", f=NF), ctx["wq3"]
                )
            for _r in range(attn_rep):
                emit_qproj(0, c)
                pump_B()
                for h in range(HPC):
                    nq = (h + 1, c) if h + 1 < HPC else None
                    emit_attn(h, c, next_qproj=nq)
        pump_B()
        flush_tail()


def _host_prep(queries, keys, Wq, bq, Wk, bk, Wv, bv, act_dt=None):
    """Build the 8 per-core input maps (host-side shard + layout prep)."""
    import ml_dtypes

    fp8 = ml_dtypes.float8_e4m3
    bf16 = ml_dtypes.bfloat16

    def to_fp8(x):
        return np.clip(x, -240.0, 240.0).astype(fp8)

    queries = np.ascontiguousarray(queries, dtype=np.float32)
    keys = np.ascontiguousarray(keys, dtype=np.float32)

    qT = np.ascontiguousarray(queries.transpose(0, 2, 1))  # [B, HID, S]
    kT = np.ascontiguousarray(keys.transpose(0, 2, 1))
    qT8 = to_fp8(qT)
    kT8 = to_fp8(kT)
    WqT = to_fp8(np.asarray(Wq, np.float32).T * WS)  # [f, e]
    WkT = to_fp8(np.asarray(Wk, np.float32).T * WS)
    WvT = to_fp8(np.asarray(Wv, np.float32).T * WS)
    bq = np.asarray(bq, np.float32) * WS
    bk = np.asarray(bk, np.float32) * WS
    bv = np.asarray(bv, np.float32) * WS

    # key padding mask -> additive bias per (b, k): 0 keep, -1e30 mask
    ksum = keys.sum(axis=-1)  # [B, S]
    kbias_all = np.where(ksum != 0.0, np.float32(0), NEG_BIAS).astype(np.float32)

    # sliding-window causal masks for the 4 diagonal-band k-tiles of a
    # q-chunk: wins[p, j*QC + q] = 1 iff q >= p + 128*j
    j_idx = np.arange(4)[None, :, None]
    wins = (
        np.arange(QC)[None, None, :] >= (np.arange(DH)[:, None, None] + DH * j_idx)
    ).astype(fp8).reshape(DH, 4 * QC)

    ones16 = np.full((DH, 32), WS, fp8)       # row-sum DR weights (value 32)
    onesr32 = np.ones((1, DH), np.float32)    # broadcast lhsT (f32r)
    onesr8 = np.ones((1, DH), fp8)            # V-bias lhsT

    in_maps = []
    for c in range(NCORES):
        b, hg = divmod(c, 2)
        e0 = hg * EH
        in_maps.append(
            {
                "qT": qT8[b],
                "kT": kT8[b],
                "wqT": np.ascontiguousarray(WqT[:, e0 : e0 + EH]),
                "wkT": np.ascontiguousarray(WkT[:, e0 : e0 + EH]),
                "wvT": np.ascontiguousarray(WvT[:, e0 : e0 + EH]),
                "bq_d": np.ascontiguousarray(bq[e0 : e0 + EH].reshape(HPC, DH).T),
                "bk_d": np.ascontiguousarray(bk[e0 : e0 + EH].reshape(HPC, DH).T),
                "bv_d": to_fp8(bv[e0 : e0 + EH].reshape(1, EH)),
                "kbias_d": np.ascontiguousarray(kbias_all[b].reshape(NKT, DH).T),
                "wins_d": wins,
                "ones16_d": ones16,
                "onesr32_d": onesr32,
                "onesr8_d": onesr8,
                "resid_d": qT[b][e0 : e0 + EH, :].astype(bf16),
            }
        )
    return in_maps


def _assemble(results):
    """results: list of 8 dicts with outT_d [EH, S] -> full [B, S, HID]."""
    out = np.empty((B, S, HID), np.float32)
    for c in range(NCORES):
        b, hg = divmod(c, 2)
        out[b, :, hg * EH : (hg + 1) * EH] = results[c]["outT_d"].T
    return out


def _flags(inputs):
    keys = np.asarray(inputs["keys"], np.float32)
    fast = not bool(np.any(keys.sum(axis=-1) == 0.0))
    zero_bias = all(
        not np.any(np.asarray(inputs[k], np.float32))
        for k in ("bq", "bk", "bv")
    )
    return fast, zero_bias


def kernel(**inputs):
    from concourse.bass_utils import run_bass_kernel_spmd

    fast, zero_bias = _flags(inputs)
    nc = _build(fast=fast, zero_bias=zero_bias)
    in_maps = _host_prep(**inputs)
    res = run_bass_kernel_spmd(nc, in_maps, core_ids=list(range(NCORES)))
    kernel.last_results = res
    return _assemble(res.results)
